# revision 5
# baseline (speedup 1.0000x reference)
"""Multi-head causal attention (B=4, T=4096, D=1024, H=16) on 8 TRN2 NeuronCores.

Sharding: core c -> (batch b = c//2, head-group g = c%2 of 8 heads).
Per core: QKV projection (fp32r matmuls), flash-style causal attention with
transposed layouts (no transposes inside attention), output projection.
Host sums the two per-batch partials (w_proj row-split) and transposes.

Internals:
  qT, kT  [512, T]   feature-on-partition layout (from W-stationary projection)
  V       [4 pairs, T, 130]  natural layout + ones column per head (denominator
                             comes out of the same AV matmul in PSUM row 64)
  S^T     [tk, tq] blocks in PSUM; exp on ScalarE (scale=0.125 folds 1/sqrt(64))
  softmax without max-subtraction (logits are ~N(0,1); exp never overflows)
  causal: block-skip above diagonal, 0/1 mask multiply on diagonal tiles
"""
import numpy as np

B, T, D = 4, 4096, 1024
H, HD = 16, 64
N_CORES = 8
PAIRS = 4            # head-pairs per core (8 local heads)
DL = PAIRS * 128     # 512 = local q/k/v width
TQ = 512             # query block
NTQ = T // TQ        # 8

_CACHE = {}


def _build_masks():
    # mask[g][p, tkb*512 + q] = 1.0 iff p + (256*g + 128*tkb) <= q
    m = np.zeros((2, 128, 1024), dtype=np.float32)
    p = np.arange(128)[:, None]
    q = np.arange(512)[None, :]
    for g in range(2):
        for tkb in range(2):
            d = 256 * g + 128 * tkb
            m[g, :, tkb * 512:(tkb + 1) * 512] = (p + d <= q).astype(np.float32)
    return m


def _build_nc():
    import concourse.tile as tile
    from concourse import bacc, mybir

    fp32 = mybir.dt.float32
    fp32r = mybir.dt.float32r
    AF = mybir.ActivationFunctionType

    nc = bacc.Bacc("TRN2", target_bir_lowering=False, debug=False,
                   num_devices=N_CORES)
    xb_d = nc.dram_tensor("xb", [T, D], fp32, kind="ExternalInput").ap()
    wqkv_d = nc.dram_tensor("wqkv", [D, 3 * DL], fp32r, kind="ExternalInput").ap()
    wp_d = nc.dram_tensor("wp", [DL, D], fp32r, kind="ExternalInput").ap()
    mask_d = nc.dram_tensor("mask", [2, 128, 1024], fp32r, kind="ExternalInput").ap()
    id_d = nc.dram_tensor("ident", [128, 128], fp32, kind="ExternalInput").ap()
    yt_d = nc.dram_tensor("yt", [D, T], fp32, kind="ExternalOutput").ap()
    qt_d = nc.dram_tensor("qt", [DL, T], fp32r)
    kt_d = nc.dram_tensor("kt", [DL, T], fp32r)
    v_d = nc.dram_tensor("v", [PAIRS, T, 130], fp32r)

    with tile.TileContext(nc) as tc:
        # ---------------- Phase A: QKV projection ----------------
        with (
            tc.tile_pool(name="pa", bufs=1) as pa,
            tc.tile_pool(name="pap", bufs=1, space="PSUM") as pap,
        ):
            ident = pa.tile([128, 128], fp32, tag="ident")
            nc.sync.dma_start(ident[:], id_d[:])
            wqkv = pa.tile([128, 8, 3 * DL], fp32r, tag="wqkv")
            nc.sync.dma_start(wqkv[:], wqkv_d.rearrange("(a p) f -> p a f", p=128))

            for tb in range(NTQ):
                x_sb = pa.tile([128, 4, D], fp32, tag="x", bufs=2)
                nc.sync.dma_start(
                    x_sb[:],
                    xb_d[tb * 512:(tb + 1) * 512, :].rearrange("(a p) f -> p a f", p=128))
                xT = pa.tile([128, 8, 512], fp32r, tag="xT", bufs=2)
                for k in range(8):
                    pxt = pap.tile([128, 512], fp32, tag="pxt", bufs=2)
                    for s in range(4):
                        nc.tensor.transpose(pxt[:, s * 128:(s + 1) * 128],
                                            x_sb[:, s, k * 128:(k + 1) * 128],
                                            ident[:])
                    nc.vector.tensor_copy(xT[:, k, :], pxt[:])
                # Q, K sections: W stationary -> transposed output [f, t]
                for fc in range(8):
                    ps = pap.tile([128, 512], fp32, tag="pqk", bufs=2)
                    for k in range(8):
                        nc.tensor.matmul(ps[:], wqkv[:, k, fc * 128:(fc + 1) * 128],
                                         xT[:, k, :], start=(k == 0), stop=(k == 7))
                    st = pa.tile([128, 512], fp32r, tag="qks", bufs=3)
                    nc.vector.tensor_copy(st[:], ps[:])
                    dst = qt_d if fc < 4 else kt_d
                    fcl = fc % 4
                    nc.sync.dma_start(
                        dst[fcl * 128:(fcl + 1) * 128, tb * 512:(tb + 1) * 512], st[:])
                # V section: xT stationary -> natural output [t, f]
                for s in range(4):
                    ps = pap.tile([128, 512], fp32, tag="pqk", bufs=2)
                    for k in range(8):
                        nc.tensor.matmul(ps[:], xT[:, k, s * 128:(s + 1) * 128],
                                         wqkv[:, k, 2 * DL:3 * DL],
                                         start=(k == 0), stop=(k == 7))
                    vs = pa.tile([128, 4, 2, 65], fp32r, tag="vst", bufs=2)
                    nc.vector.memset(vs[:].bitcast(fp32), 1.0)
                    nc.vector.tensor_copy(
                        vs[:, :, :, 0:64],
                        ps[:].rearrange("p (a h e) -> p a h e", a=4, h=2))
                    r0 = tb * 512 + s * 128
                    nc.sync.dma_start(
                        v_d[:, r0:r0 + 128, :].rearrange("a p f -> p a f"),
                        vs[:].rearrange("p a h e -> p a (h e)"))

        # ---------------- Phase B+C: attention + projection ----------------
        with (
            tc.tile_pool(name="pb", bufs=1) as pb,
            tc.tile_pool(name="pbp", bufs=1, space="PSUM") as pbp,
        ):
            wp = pb.tile([128, 4, D], fp32r, tag="wp")
            nc.sync.dma_start(wp[:], wp_d.rearrange("(a p) f -> p a f", p=128))
            msk = pb.tile([128, 2, 1024], fp32r, tag="msk")
            nc.sync.dma_start(msk[:], mask_d.rearrange("g p f -> p g f"))

            for j in range(NTQ):
                otsb = [pb.tile([128, TQ], fp32r, tag=f"otsb{pr}", bufs=2, name=f"otsb{pr}_{j}")
                        for pr in range(PAIRS)]
                for pr in range(PAIRS):
                    nkb = 4 * (j + 1)       # causal tk blocks for this row
                    kt = pb.tile([128, T], fp32r, tag="kt", bufs=2)
                    nc.sync.dma_start(kt[:, :512 * (j + 1)],
                                      kt_d[pr * 128:(pr + 1) * 128, :512 * (j + 1)])
                    qt = pb.tile([128, TQ], fp32r, tag="qt", bufs=2)
                    nc.sync.dma_start(qt[:],
                                      qt_d[pr * 128:(pr + 1) * 128, j * 512:(j + 1) * 512])
                    vt = pb.tile([128, 32, 130], fp32r, tag="vt", bufs=2)
                    nc.sync.dma_start(
                        vt[:, :nkb, :],
                        v_d[pr, :512 * (j + 1), :].rearrange("(a p) f -> p a f", p=128))
                    otacc = [pb.tile([65, TQ], fp32, tag=f"otacc{h_}", bufs=2,
                                     name=f"otacc{h_}_{pr}_{j}")
                             for h_ in range(2)]
                    for c in range(j + 1):
                        ot = [pbp.tile([65, TQ], fp32, tag="ot", bufs=2,
                                       name=f"ot{h_}_{pr}_{j}_{c}")
                              for h_ in range(2)]
                        for g in range(2):
                            sts = [pbp.tile([128, 1024], fp32, tag="st", bufs=3, name=f"st{h_}_{pr}_{j}_{c}_{g}")
                                   for h_ in range(2)]
                            for tkb in range(2):
                                blk = c * 4 + g * 2 + tkb
                                for h in range(2):
                                    nc.tensor.matmul(
                                        sts[h][:, tkb * 512:(tkb + 1) * 512],
                                        kt[h * 64:(h + 1) * 64, blk * 128:(blk + 1) * 128],
                                        qt[h * 64:(h + 1) * 64, :],
                                        start=True, stop=True)
                            for h in range(2):
                                ex = pb.tile([128, 1024], fp32r, tag="ex", bufs=4)
                                nc.scalar.activation(ex[:], sts[h][:], AF.Exp,
                                                     scale=0.125)
                                if c == j:
                                    nc.vector.tensor_mul(ex[:], ex[:], msk[:, g, :])
                                for tkb in range(2):
                                    blk = c * 4 + g * 2 + tkb
                                    nc.tensor.matmul(
                                        ot[h][:],
                                        vt[:, blk, h * 65:(h + 1) * 65],
                                        ex[:, tkb * 512:(tkb + 1) * 512],
                                        start=(g == 0 and tkb == 0),
                                        stop=(g == 1 and tkb == 1))
                        for h in range(2):
                            if c == 0:
                                nc.vector.tensor_copy(otacc[h][:], ot[h][:])
                            else:
                                nc.vector.tensor_add(otacc[h][:], otacc[h][:], ot[h][:])
                    for h in range(2):
                        bc = pb.tile([64, TQ], fp32, tag="bc", bufs=2)
                        nc.gpsimd.partition_broadcast(bc[:], otacc[h][64:65, :])
                        rec = pb.tile([64, TQ], fp32, tag="rec", bufs=2)
                        nc.vector.reciprocal(rec[:], bc[:])
                        nc.vector.tensor_mul(otsb[pr][h * 64:(h + 1) * 64, :],
                                             otacc[h][0:64, :], rec[:])
                # output projection for this tq block
                for mc in range(8):
                    yp = pbp.tile([128, TQ], fp32, tag="ot", bufs=2)
                    for kc in range(PAIRS):
                        nc.tensor.matmul(yp[:], wp[:, kc, mc * 128:(mc + 1) * 128],
                                         otsb[kc][:],
                                         start=(kc == 0), stop=(kc == PAIRS - 1))
                    ys = pb.tile([128, TQ], fp32, tag="ys", bufs=3)
                    nc.vector.tensor_copy(ys[:], yp[:])
                    nc.sync.dma_start(
                        yt_d[mc * 128:(mc + 1) * 128, j * 512:(j + 1) * 512], ys[:])

    nc.compile()
    return nc


def _get_nc():
    if "nc" not in _CACHE:
        _CACHE["nc"] = _build_nc()
    return _CACHE["nc"]


def _in_maps(x, w_qkv, w_proj):
    masks = _build_masks()
    ident = np.eye(128, dtype=np.float32)
    maps = []
    for c in range(N_CORES):
        b, g = c // 2, c % 2
        wq = w_qkv[:, g * DL:(g + 1) * DL]
        wk = w_qkv[:, D + g * DL:D + (g + 1) * DL]
        wv = w_qkv[:, 2 * D + g * DL:2 * D + (g + 1) * DL]
        maps.append({
            "xb": np.ascontiguousarray(x[b]),
            "wqkv": np.ascontiguousarray(np.concatenate([wq, wk, wv], axis=1)),
            "wp": np.ascontiguousarray(w_proj[g * DL:(g + 1) * DL, :]),
            "mask": masks,
            "ident": ident,
        })
    return maps


def _run(x, w_qkv, w_proj, trace=False):
    from concourse.bass_utils import run_bass_kernel_spmd

    nc = _get_nc()
    res = run_bass_kernel_spmd(nc, _in_maps(x, w_qkv, w_proj),
                               core_ids=list(range(N_CORES)), trace=trace)
    outs = [res.results[c]["yt"] for c in range(N_CORES)]
    y = np.stack([(outs[2 * b] + outs[2 * b + 1]).T for b in range(B)])
    return np.ascontiguousarray(y.astype(np.float32)), res


def kernel(x, w_qkv, w_proj):
    x = np.asarray(x, dtype=np.float32)
    w_qkv = np.asarray(w_qkv, dtype=np.float32)
    w_proj = np.asarray(w_proj, dtype=np.float32)
    y, _ = _run(x, w_qkv, w_proj, trace=False)
    return y


def kernel_traced(x, w_qkv, w_proj):
    import prof_shim
    prof_shim.install()
    y, res = _run(np.asarray(x, np.float32), np.asarray(w_qkv, np.float32),
                  np.asarray(w_proj, np.float32), trace=True)
    return y, res


# revision 6
# speedup vs baseline: 1.2879x; 1.2879x over previous
"""Multi-head causal attention (B=4, T=4096, D=1024, H=16) on 8 TRN2 NeuronCores.

Sharding: core c -> (batch b = c//2, head-group g = c%2 of 8 heads).
Per core: QKV projection (fp32r matmuls), flash-style causal attention with
transposed layouts (no transposes inside attention), output projection.
Host sums the two per-batch partials (w_proj row-split) and transposes.

Internals:
  qT, kT  [512, T]   feature-on-partition layout (from W-stationary projection)
  V       [4 pairs, T, 130]  natural layout + ones column per head (denominator
                             comes out of the same AV matmul in PSUM row 64)
  S^T     [tk, tq] blocks in PSUM; exp on ScalarE (scale=0.125 folds 1/sqrt(64))
  softmax without max-subtraction (logits are ~N(0,1); exp never overflows)
  causal: block-skip above diagonal, 0/1 mask multiply on diagonal tiles
"""
import numpy as np

B, T, D = 4, 4096, 1024
H, HD = 16, 64
N_CORES = 8
PAIRS = 4            # head-pairs per core (8 local heads)
DL = PAIRS * 128     # 512 = local q/k/v width
TQ = 512             # query block
NTQ = T // TQ        # 8

_CACHE = {}


def _build_masks():
    # mask[g][p, tkb*512 + q] = 1.0 iff p + (256*g + 128*tkb) <= q
    m = np.zeros((2, 128, 1024), dtype=np.float32)
    p = np.arange(128)[:, None]
    q = np.arange(512)[None, :]
    for g in range(2):
        for tkb in range(2):
            d = 256 * g + 128 * tkb
            m[g, :, tkb * 512:(tkb + 1) * 512] = (p + d <= q).astype(np.float32)
    return m


def _build_nc():
    import concourse.tile as tile
    from concourse import bacc, mybir

    fp32 = mybir.dt.float32
    fp32r = mybir.dt.float32r
    AF = mybir.ActivationFunctionType

    nc = bacc.Bacc("TRN2", target_bir_lowering=False, debug=False,
                   num_devices=N_CORES)
    xb_d = nc.dram_tensor("xb", [T, D], fp32, kind="ExternalInput").ap()
    wqkv_d = nc.dram_tensor("wqkv", [D, 3 * DL], fp32r, kind="ExternalInput").ap()
    wp_d = nc.dram_tensor("wp", [DL, D], fp32r, kind="ExternalInput").ap()
    mask_d = nc.dram_tensor("mask", [2, 128, 1024], fp32r, kind="ExternalInput").ap()
    id_d = nc.dram_tensor("ident", [128, 128], fp32, kind="ExternalInput").ap()
    yt_d = nc.dram_tensor("yt", [D, T], fp32, kind="ExternalOutput").ap()
    qt_d = nc.dram_tensor("qt", [DL, T], fp32r)
    kt_d = nc.dram_tensor("kt", [DL, T], fp32r)
    v_d = nc.dram_tensor("v", [PAIRS, T, 130], fp32r)

    with tile.TileContext(nc) as tc:
        # ---------------- Phase A: QKV projection ----------------
        with (
            tc.tile_pool(name="pa", bufs=1) as pa,
            tc.tile_pool(name="pap", bufs=1, space="PSUM") as pap,
        ):
            ident = pa.tile([128, 128], fp32, tag="ident")
            nc.sync.dma_start(ident[:], id_d[:])
            wqkv = pa.tile([128, 8, 3 * DL], fp32r, tag="wqkv")
            nc.sync.dma_start(wqkv[:], wqkv_d.rearrange("(a p) f -> p a f", p=128))

            for tb in range(NTQ):
                x_sb = pa.tile([128, 4, D], fp32, tag="x", bufs=2)
                nc.sync.dma_start(
                    x_sb[:],
                    xb_d[tb * 512:(tb + 1) * 512, :].rearrange("(a p) f -> p a f", p=128))
                xT = pa.tile([128, 8, 512], fp32r, tag="xT", bufs=2)
                for k in range(8):
                    pxt = pap.tile([128, 512], fp32, tag="pxt", bufs=2)
                    for s in range(4):
                        nc.tensor.transpose(pxt[:, s * 128:(s + 1) * 128],
                                            x_sb[:, s, k * 128:(k + 1) * 128],
                                            ident[:])
                    nc.vector.tensor_copy(xT[:, k, :], pxt[:])
                # Q, K sections: W stationary -> transposed output [f, t]
                for fc in range(8):
                    ps = pap.tile([128, 512], fp32, tag="pqk", bufs=2)
                    for k in range(8):
                        nc.tensor.matmul(ps[:], wqkv[:, k, fc * 128:(fc + 1) * 128],
                                         xT[:, k, :], start=(k == 0), stop=(k == 7))
                    st = pa.tile([128, 512], fp32r, tag="qks", bufs=3)
                    nc.vector.tensor_copy(st[:], ps[:])
                    dst = qt_d if fc < 4 else kt_d
                    fcl = fc % 4
                    nc.sync.dma_start(
                        dst[fcl * 128:(fcl + 1) * 128, tb * 512:(tb + 1) * 512], st[:])
                # V section: xT stationary -> natural output [t, f]
                for s in range(4):
                    ps = pap.tile([128, 512], fp32, tag="pqk", bufs=2)
                    for k in range(8):
                        nc.tensor.matmul(ps[:], xT[:, k, s * 128:(s + 1) * 128],
                                         wqkv[:, k, 2 * DL:3 * DL],
                                         start=(k == 0), stop=(k == 7))
                    vs = pa.tile([128, 4, 2, 65], fp32r, tag="vst", bufs=2)
                    nc.vector.memset(vs[:].bitcast(fp32), 1.0)
                    nc.vector.tensor_copy(
                        vs[:, :, :, 0:64],
                        ps[:].rearrange("p (a h e) -> p a h e", a=4, h=2))
                    r0 = tb * 512 + s * 128
                    nc.sync.dma_start(
                        v_d[:, r0:r0 + 128, :].rearrange("a p f -> p a f"),
                        vs[:].rearrange("p a h e -> p a (h e)"))

        # ---------------- Phase B+C: attention + projection ----------------
        with (
            tc.tile_pool(name="pb", bufs=1) as pb,
            tc.tile_pool(name="pbp", bufs=1, space="PSUM") as pbp,
        ):
            wp = pb.tile([128, 4, D], fp32r, tag="wp")
            nc.sync.dma_start(wp[:], wp_d.rearrange("(a p) f -> p a f", p=128))
            msk = pb.tile([128, 2, 1024], fp32r, tag="msk")
            nc.sync.dma_start(msk[:], mask_d.rearrange("g p f -> p g f"))

            for j in range(NTQ):
                otsb = [pb.tile([128, TQ], fp32r, tag=f"otsb{pr}", bufs=2, name=f"otsb{pr}_{j}")
                        for pr in range(PAIRS)]
                for pr in range(PAIRS):
                    nkb = 4 * (j + 1)       # causal tk blocks for this row
                    kt = pb.tile([128, T], fp32r, tag="kt", bufs=2)
                    nc.sync.dma_start(kt[:, :512 * (j + 1)],
                                      kt_d[pr * 128:(pr + 1) * 128, :512 * (j + 1)])
                    qt = pb.tile([128, TQ], fp32r, tag="qt", bufs=2)
                    nc.sync.dma_start(qt[:],
                                      qt_d[pr * 128:(pr + 1) * 128, j * 512:(j + 1) * 512])
                    vt = pb.tile([128, 32, 130], fp32r, tag="vt", bufs=2)
                    nc.sync.dma_start(
                        vt[:, :nkb, :],
                        v_d[pr, :512 * (j + 1), :].rearrange("(a p) f -> p a f", p=128))
                    ot = [pbp.tile([65, TQ], fp32, tag="ot", bufs=2, name=f"ot{h_}_{pr}_{j}")
                          for h_ in range(2)]
                    for c in range(j + 1):
                        for g in range(2):
                            sts = [pbp.tile([128, 1024], fp32, tag="st", bufs=2, name=f"st{h_}_{pr}_{j}_{c}_{g}")
                                   for h_ in range(2)]
                            for tkb in range(2):
                                blk = c * 4 + g * 2 + tkb
                                for h in range(2):
                                    nc.tensor.matmul(
                                        sts[h][:, tkb * 512:(tkb + 1) * 512],
                                        kt[h * 64:(h + 1) * 64, blk * 128:(blk + 1) * 128],
                                        qt[h * 64:(h + 1) * 64, :],
                                        start=True, stop=True)
                            for h in range(2):
                                ex = pb.tile([128, 1024], fp32r, tag="ex", bufs=4)
                                nc.scalar.activation(ex[:], sts[h][:], AF.Exp,
                                                     scale=0.125)
                                if c == j:
                                    nc.vector.tensor_mul(ex[:], ex[:], msk[:, g, :])
                                for tkb in range(2):
                                    blk = c * 4 + g * 2 + tkb
                                    nc.tensor.matmul(
                                        ot[h][:],
                                        vt[:, blk, h * 65:(h + 1) * 65],
                                        ex[:, tkb * 512:(tkb + 1) * 512],
                                        start=(c == 0 and g == 0 and tkb == 0),
                                        stop=(c == j and g == 1 and tkb == 1))
                    for h in range(2):
                        rec = pb.tile([1, TQ], fp32, tag="rec", bufs=2)
                        nc.vector.reciprocal_approx_fast(rec[:], ot[h][64:65, :])
                        bc = pb.tile([64, TQ], fp32, tag="bc", bufs=2)
                        nc.gpsimd.partition_broadcast(bc[:], rec[:])
                        nc.vector.tensor_mul(otsb[pr][h * 64:(h + 1) * 64, :],
                                             ot[h][0:64, :], bc[:])
                # output projection for this tq block
                for mc in range(8):
                    yp = pbp.tile([128, TQ], fp32, tag="yp", bufs=2)
                    for kc in range(PAIRS):
                        nc.tensor.matmul(yp[:], wp[:, kc, mc * 128:(mc + 1) * 128],
                                         otsb[kc][:],
                                         start=(kc == 0), stop=(kc == PAIRS - 1))
                    ys = pb.tile([128, TQ], fp32, tag="ys", bufs=3)
                    nc.vector.tensor_copy(ys[:], yp[:])
                    nc.sync.dma_start(
                        yt_d[mc * 128:(mc + 1) * 128, j * 512:(j + 1) * 512], ys[:])

    nc.compile()
    return nc


def _get_nc():
    if "nc" not in _CACHE:
        _CACHE["nc"] = _build_nc()
    return _CACHE["nc"]


def _in_maps(x, w_qkv, w_proj):
    masks = _build_masks()
    ident = np.eye(128, dtype=np.float32)
    maps = []
    for c in range(N_CORES):
        b, g = c // 2, c % 2
        wq = w_qkv[:, g * DL:(g + 1) * DL]
        wk = w_qkv[:, D + g * DL:D + (g + 1) * DL]
        wv = w_qkv[:, 2 * D + g * DL:2 * D + (g + 1) * DL]
        maps.append({
            "xb": np.ascontiguousarray(x[b]),
            "wqkv": np.ascontiguousarray(np.concatenate([wq, wk, wv], axis=1)),
            "wp": np.ascontiguousarray(w_proj[g * DL:(g + 1) * DL, :]),
            "mask": masks,
            "ident": ident,
        })
    return maps


def _run(x, w_qkv, w_proj, trace=False):
    from concourse.bass_utils import run_bass_kernel_spmd

    nc = _get_nc()
    res = run_bass_kernel_spmd(nc, _in_maps(x, w_qkv, w_proj),
                               core_ids=list(range(N_CORES)), trace=trace)
    outs = [res.results[c]["yt"] for c in range(N_CORES)]
    y = np.stack([(outs[2 * b] + outs[2 * b + 1]).T for b in range(B)])
    return np.ascontiguousarray(y.astype(np.float32)), res


def kernel(x, w_qkv, w_proj):
    x = np.asarray(x, dtype=np.float32)
    w_qkv = np.asarray(w_qkv, dtype=np.float32)
    w_proj = np.asarray(w_proj, dtype=np.float32)
    y, _ = _run(x, w_qkv, w_proj, trace=False)
    return y


def kernel_traced(x, w_qkv, w_proj):
    import prof_shim
    prof_shim.install()
    y, res = _run(np.asarray(x, np.float32), np.asarray(w_qkv, np.float32),
                  np.asarray(w_proj, np.float32), trace=True)
    return y, res


# revision 9
# speedup vs baseline: 1.5165x; 1.1775x over previous
"""Multi-head causal attention (B=4, T=4096, D=1024, H=16) on 8 TRN2 NeuronCores.

Sharding: core c -> (batch b = c//2, head-group g = c%2 of 8 heads).
Per core: QKV projection (fp32r matmuls), flash-style causal attention with
transposed layouts (no transposes inside attention), output projection.
Host sums the two per-batch partials (w_proj row-split) and transposes.

Internals:
  qT, kT  [512, T]   feature-on-partition layout (from W-stationary projection)
  V       [4 pairs, T, 130]  natural layout + ones column per head (denominator
                             comes out of the same AV matmul in PSUM row 64)
  S^T     [tk, tq] blocks in PSUM; exp on ScalarE (scale=0.125 folds 1/sqrt(64))
  softmax without max-subtraction (logits are ~N(0,1); exp never overflows)
  causal: block-skip above diagonal, 0/1 mask multiply on diagonal tiles
  phase interleaving: QKV-projection block tb+1 is emitted between attention
  rows so its PE work fills the ScalarE(exp)-wait gaps and keeps the PE warm
"""
import numpy as np

B, T, D = 4, 4096, 1024
H, HD = 16, 64
N_CORES = 8
PAIRS = 4            # head-pairs per core (8 local heads)
DL = PAIRS * 128     # 512 = local q/k/v width
TQ = 512             # query block
NTQ = T // TQ        # 8

_CACHE = {}


def _build_masks():
    # mask[g][p, tkb*512 + q] = 1.0 iff p + (256*g + 128*tkb) <= q
    m = np.zeros((2, 128, 1024), dtype=np.float32)
    p = np.arange(128)[:, None]
    q = np.arange(512)[None, :]
    for g in range(2):
        for tkb in range(2):
            d = 256 * g + 128 * tkb
            m[g, :, tkb * 512:(tkb + 1) * 512] = (p + d <= q).astype(np.float32)
    return m


def _build_nc():
    import concourse.tile as tile
    from concourse import bacc, mybir

    fp32 = mybir.dt.float32
    fp32r = mybir.dt.float32r
    AF = mybir.ActivationFunctionType

    nc = bacc.Bacc("TRN2", target_bir_lowering=False, debug=False,
                   num_devices=N_CORES)
    xb_d = nc.dram_tensor("xb", [T, D], fp32, kind="ExternalInput").ap()
    wqkv_d = nc.dram_tensor("wqkv", [D, 3 * DL], fp32r, kind="ExternalInput").ap()
    wp_d = nc.dram_tensor("wp", [DL, D], fp32r, kind="ExternalInput").ap()
    mask_d = nc.dram_tensor("mask", [2, 128, 1024], fp32r, kind="ExternalInput").ap()
    id_d = nc.dram_tensor("ident", [128, 128], fp32, kind="ExternalInput").ap()
    yt_d = nc.dram_tensor("yt", [D, T], fp32, kind="ExternalOutput").ap()
    qt_d = nc.dram_tensor("qt", [DL, T], fp32r)
    kt_d = nc.dram_tensor("kt", [DL, T], fp32r)
    v_d = nc.dram_tensor("v", [PAIRS, T, 130], fp32r)

    with tile.TileContext(nc) as tc:
        with (
            tc.tile_pool(name="sb", bufs=1) as pool,
            tc.tile_pool(name="ps", bufs=1, space="PSUM") as psum,
        ):
            ident = pool.tile([128, 128], fp32, tag="ident")
            nc.sync.dma_start(ident[:], id_d[:])
            wqkv = pool.tile([128, 8, 3 * DL], fp32r, tag="wqkv")
            nc.sync.dma_start(wqkv[:], wqkv_d.rearrange("(a p) f -> p a f", p=128))
            wp = pool.tile([128, 4, D], fp32r, tag="wp")
            nc.sync.dma_start(wp[:], wp_d.rearrange("(a p) f -> p a f", p=128))
            msk = pool.tile([128, 2, 1024], fp32r, tag="msk")
            nc.sync.dma_start(msk[:], mask_d.rearrange("g p f -> p g f"))

            def emit_proj_block(tb):
                """QKV projection for t rows [tb*512, (tb+1)*512)."""
                x_sb = pool.tile([128, 4, D], fp32, tag="x", bufs=2,
                                 name=f"x_{tb}")
                nc.sync.dma_start(
                    x_sb[:],
                    xb_d[tb * 512:(tb + 1) * 512, :]
                    .rearrange("(a p) f -> p a f", p=128))
                xT = pool.tile([128, 8, 512], fp32r, tag="xT", bufs=2,
                               name=f"xT_{tb}")
                for k in range(8):
                    pxt = psum.tile([128, 512], fp32, tag="yp", bufs=2,
                                    name=f"pxt_{tb}_{k}")
                    for s in range(4):
                        nc.tensor.transpose(pxt[:, s * 128:(s + 1) * 128],
                                            x_sb[:, s, k * 128:(k + 1) * 128],
                                            ident[:])
                    nc.vector.tensor_copy(xT[:, k, :], pxt[:])
                # Q, K sections: W stationary -> transposed output [f, t]
                for fc in range(8):
                    ps = psum.tile([128, 512], fp32, tag="yp", bufs=2,
                                   name=f"pqk_{tb}_{fc}")
                    for k in range(8):
                        nc.tensor.matmul(ps[:], wqkv[:, k, fc * 128:(fc + 1) * 128],
                                         xT[:, k, :], start=(k == 0), stop=(k == 7))
                    st = pool.tile([128, 512], fp32r, tag="qks", bufs=2,
                                   name=f"qks_{tb}_{fc}")
                    nc.vector.tensor_copy(st[:], ps[:])
                    dst = qt_d if fc < 4 else kt_d
                    fcl = fc % 4
                    nc.sync.dma_start(
                        dst[fcl * 128:(fcl + 1) * 128, tb * 512:(tb + 1) * 512],
                        st[:])
                # V section: xT stationary -> natural output [t, f]
                for s in range(4):
                    ps = psum.tile([128, 512], fp32, tag="yp", bufs=2,
                                   name=f"pv_{tb}_{s}")
                    for k in range(8):
                        nc.tensor.matmul(ps[:], xT[:, k, s * 128:(s + 1) * 128],
                                         wqkv[:, k, 2 * DL:3 * DL],
                                         start=(k == 0), stop=(k == 7))
                    vs = pool.tile([128, 4, 2, 65], fp32r, tag="vst", bufs=2,
                                   name=f"vst_{tb}_{s}")
                    nc.vector.memset(vs[:].bitcast(fp32), 1.0)
                    nc.vector.tensor_copy(
                        vs[:, :, :, 0:64],
                        ps[:].rearrange("p (a h e) -> p a h e", a=4, h=2))
                    r0 = tb * 512 + s * 128
                    nc.sync.dma_start(
                        v_d[:, r0:r0 + 128, :].rearrange("a p f -> p a f"),
                        vs[:].rearrange("p a h e -> p a (h e)"))

            def emit_attn_row(j):
                """Attention + output projection for tq rows [j*512, (j+1)*512)."""
                otsb = [pool.tile([128, TQ], fp32r, tag=f"otsb{pr}", bufs=2,
                                  name=f"otsb{pr}_{j}")
                        for pr in range(PAIRS)]
                for pr in range(PAIRS):
                    qt = pool.tile([128, TQ], fp32r, tag="qt", bufs=2,
                                   name=f"qt_{pr}_{j}")
                    nc.sync.dma_start(
                        qt[:],
                        qt_d[pr * 128:(pr + 1) * 128, j * 512:(j + 1) * 512])
                    ot = [psum.tile([65, TQ], fp32, tag="ot", bufs=2,
                                    name=f"ot{h_}_{pr}_{j}")
                          for h_ in range(2)]
                    for c in range(j + 1):
                        kt = pool.tile([128, TQ], fp32r, tag="kt", bufs=3,
                                       name=f"kt_{pr}_{j}_{c}")
                        nc.sync.dma_start(
                            kt[:],
                            kt_d[pr * 128:(pr + 1) * 128, c * 512:(c + 1) * 512])
                        vt = pool.tile([128, 4, 130], fp32r, tag="vt", bufs=3,
                                       name=f"vt_{pr}_{j}_{c}")
                        nc.sync.dma_start(
                            vt[:],
                            v_d[pr, c * 512:(c + 1) * 512, :]
                            .rearrange("(a p) f -> p a f", p=128))
                        for g in range(2):
                            sts = [psum.tile([128, 1024], fp32, tag="st", bufs=2,
                                             name=f"st{h_}_{pr}_{j}_{c}_{g}")
                                   for h_ in range(2)]
                            for tkb in range(2):
                                for h in range(2):
                                    nc.tensor.matmul(
                                        sts[h][:, tkb * 512:(tkb + 1) * 512],
                                        kt[h * 64:(h + 1) * 64,
                                           (g * 2 + tkb) * 128:(g * 2 + tkb + 1) * 128],
                                        qt[h * 64:(h + 1) * 64, :],
                                        start=True, stop=True)
                            for h in range(2):
                                ex = pool.tile([128, 1024], fp32r, tag="ex", bufs=4,
                                               name=f"ex{h}_{pr}_{j}_{c}_{g}")
                                nc.scalar.activation(ex[:], sts[h][:], AF.Exp,
                                                     scale=0.125)
                                if c == j:
                                    nc.vector.tensor_mul(ex[:], ex[:], msk[:, g, :])
                                for tkb in range(2):
                                    nc.tensor.matmul(
                                        ot[h][:],
                                        vt[:, g * 2 + tkb, h * 65:(h + 1) * 65],
                                        ex[:, tkb * 512:(tkb + 1) * 512],
                                        start=(c == 0 and g == 0 and tkb == 0),
                                        stop=(c == j and g == 1 and tkb == 1))
                    for h in range(2):
                        den = pool.tile([1, TQ], fp32, tag="den", bufs=1,
                                        name=f"den{h}_{pr}_{j}")
                        nc.vector.tensor_copy(den[:], ot[h][64:65, :])
                        bc = pool.tile([64, TQ], fp32, tag="bc", bufs=1,
                                       name=f"bc{h}_{pr}_{j}")
                        nc.gpsimd.partition_broadcast(bc[:], den[:])
                        rec = pool.tile([64, TQ], fp32, tag="rec", bufs=1,
                                        name=f"rec{h}_{pr}_{j}")
                        nc.vector.reciprocal_approx_fast(rec[:], bc[:])
                        nc.vector.tensor_mul(otsb[pr][h * 64:(h + 1) * 64, :],
                                             ot[h][0:64, :], rec[:])
                # output projection for this tq block
                for mc in range(8):
                    yp = psum.tile([128, TQ], fp32, tag="yp", bufs=2,
                                   name=f"yp_{j}_{mc}")
                    for kc in range(PAIRS):
                        nc.tensor.matmul(yp[:], wp[:, kc, mc * 128:(mc + 1) * 128],
                                         otsb[kc][:],
                                         start=(kc == 0), stop=(kc == PAIRS - 1))
                    ys = pool.tile([128, TQ], fp32, tag="ys", bufs=2,
                                   name=f"ys_{j}_{mc}")
                    nc.vector.tensor_copy(ys[:], yp[:])
                    nc.sync.dma_start(
                        yt_d[mc * 128:(mc + 1) * 128, j * 512:(j + 1) * 512],
                        ys[:])

            # Interleave: proj block tb+1 emitted between attention rows so
            # PE-heavy projection work overlaps ScalarE-heavy attention rows.
            emit_proj_block(0)
            emit_proj_block(1)
            for j in range(NTQ):
                if j + 2 < NTQ:
                    emit_proj_block(j + 2)
                emit_attn_row(j)

    nc.compile()
    return nc


def _get_nc():
    if "nc" not in _CACHE:
        _CACHE["nc"] = _build_nc()
    return _CACHE["nc"]


def _in_maps(x, w_qkv, w_proj):
    masks = _build_masks()
    ident = np.eye(128, dtype=np.float32)
    maps = []
    for c in range(N_CORES):
        b, g = c // 2, c % 2
        wq = w_qkv[:, g * DL:(g + 1) * DL]
        wk = w_qkv[:, D + g * DL:D + (g + 1) * DL]
        wv = w_qkv[:, 2 * D + g * DL:2 * D + (g + 1) * DL]
        maps.append({
            "xb": np.ascontiguousarray(x[b]),
            "wqkv": np.ascontiguousarray(np.concatenate([wq, wk, wv], axis=1)),
            "wp": np.ascontiguousarray(w_proj[g * DL:(g + 1) * DL, :]),
            "mask": masks,
            "ident": ident,
        })
    return maps


def _run(x, w_qkv, w_proj, trace=False):
    from concourse.bass_utils import run_bass_kernel_spmd

    nc = _get_nc()
    res = run_bass_kernel_spmd(nc, _in_maps(x, w_qkv, w_proj),
                               core_ids=list(range(N_CORES)), trace=trace)
    outs = [res.results[c]["yt"] for c in range(N_CORES)]
    y = np.stack([(outs[2 * b] + outs[2 * b + 1]).T for b in range(B)])
    return np.ascontiguousarray(y.astype(np.float32)), res


def kernel(x, w_qkv, w_proj):
    x = np.asarray(x, dtype=np.float32)
    w_qkv = np.asarray(w_qkv, dtype=np.float32)
    w_proj = np.asarray(w_proj, dtype=np.float32)
    y, _ = _run(x, w_qkv, w_proj, trace=False)
    return y


def kernel_traced(x, w_qkv, w_proj):
    import prof_shim
    prof_shim.install()
    y, res = _run(np.asarray(x, np.float32), np.asarray(w_qkv, np.float32),
                  np.asarray(w_proj, np.float32), trace=True)
    return y, res
